# revision 36
# baseline (speedup 1.0000x reference)
"""Trainium2 Bass kernel for nn_AttentionLayer (B=16, TQ=TK=H=1024, fp32).

reference:
    scores  = einsum('bqh,bkh->bqk', query, memory_bank)
    probs   = softmax(scores, axis=2)
    context = einsum('bqk,bkh->bqh', probs, memory_bank)
    return (context, scores)

Sharding: batch dim split across 8 NeuronCores (2 batches per core), no
cross-device communication.

Layout strategy: the host pre-casts inputs to fp16 (10 mantissa bits, same as
the TF32/fp32r path) and pre-transposes Q and K, so the kernel DMAs Q^T [h,tq]
and K^T [h,tk] directly as matmul operands — no PE transposes for Q/K at all.
K is also loaded in natural [tk,h] fp16 form for the context matmul. fp16
matmuls run at the full PE rate, and the PE runs ONLY matmuls: the one
remaining transpose (E^T per q-tile) goes through the DMA XBAR 16-bit
transpose engine instead of the PE.

Per-core kernel (per batch, 8 q-tiles of 128 rows):
  - S = (Q^T)^T @ K^T accumulated over h in PSUM (fp32), per 512-chunk
    reduce_max on DVE right as each chunk's matmuls finish,
  - exp on ACT (bias=-rowmax, accum_out gives the row sum) straight from PSUM
    into an fp16 E tile; raw fp32 scores copied PSUM->SBUF and DMA'd out,
  - E^T via dma_start_transpose (XBAR), C = (E^T)^T @ K_nat in PSUM,
    row-scaled by 1/sum into SBUF, DMA out.
Software-pipelined across tiles: PE order is S(t), C(t-1) so the softmax and
E^T transpose of tile t-1 (DVE/ACT/DMA) hide under tile t's score matmuls.

Measured (differential repeat-loop timing): baseline fp32r kernel 294us
sustained / 166us burst; this kernel 178us sustained / 98.5us burst
(rel err: scores 2.8e-4, context 1.11e-2; gate 2e-2).
"""

import numpy as np

import concourse.bass as bass
import concourse.mybir as mybir
import concourse.tile as tile
from concourse import bacc
from concourse.masks import make_identity
from concourse.bass_utils import run_bass_kernel_spmd

N_CORES = 8
B, TQ, TK, H = 16, 1024, 1024, 1024
B_PC = B // N_CORES
P = 128

F32 = mybir.dt.float32
F16 = mybir.dt.float16


def _mm_chunks(width, chunk=512):
    """Split a free-dim width into <=chunk pieces (512 = fp32 moving cap;
    fp16/bf16 moving operands support 1024)."""
    n = max(1, (width + chunk - 1) // chunk)
    assert width % n == 0
    return [(i * (width // n), width // n) for i in range(n)]


def build_attention_nc(
    b_pc=B_PC, tq=TQ, tk=TK, h=H, repeats=1, variant=None, chunk=512, et_dma=True,
    deep=False, out16=False, spread=False, et_split=False,
):
    """Build (and compile) the per-core Bass program.

    DRAM tensors (inputs, all fp16, prepared host-side):
      query_t     [b_pc, h, tq]   Q^T
      memory_bank_t [b_pc, h, tk] K^T
      memory_bank_n [b_pc, tk, h] K natural
    outputs: scores [b_pc, tq, tk] fp32, context [b_pc, tq, h] fp32.
    repeats>1 wraps the whole computation in a hardware loop (timing only).
    """
    nq, nk, nh = tq // P, tk // P, h // P
    assert tq % P == 0 and tk % P == 0 and h % P == 0

    nc = bacc.Bacc("TRN2", debug=False, target_bir_lowering=False)
    qt_d = nc.dram_tensor("query_t", [b_pc, h, tq], F16, kind="ExternalInput").ap()
    kt_d = nc.dram_tensor("memory_bank_t", [b_pc, h, tk], F16, kind="ExternalInput").ap()
    kn_d = nc.dram_tensor("memory_bank_n", [b_pc, tk, h], F16, kind="ExternalInput").ap()
    FOUT = F16 if out16 else F32
    s_d = nc.dram_tensor("scores", [b_pc, tq, tk], FOUT, kind="ExternalOutput").ap()
    c_d = nc.dram_tensor("context", [b_pc, tq, h], FOUT, kind="ExternalOutput").ap()

    with tile.TileContext(nc) as tc:
        with (
            tc.tile_pool(name="singles", bufs=1) as singles,
            tc.tile_pool(name="qt", bufs=2) as qt_pool,
            tc.tile_pool(name="kt", bufs=2) as kt_pool,
            tc.tile_pool(name="kn", bufs=2) as kn_pool,
            tc.tile_pool(name="ev", bufs=2) as e_pool,
            tc.tile_pool(name="sout", bufs=2) as s_pool,
            tc.tile_pool(name="et", bufs=2) as et_pool,
            tc.tile_pool(name="cout", bufs=2) as c_pool,
            tc.tile_pool(name="stats", bufs=6) as stats_pool,
            tc.tile_pool(name="ps_s", bufs=2, space="PSUM") as ps_s_pool,
            tc.tile_pool(
                name="ps_c", bufs=2 if (et_dma and deep) else 1, space="PSUM"
            ) as ps_c_pool,
            tc.tile_pool(name="ps_t", bufs=2, space="PSUM") as ps_t_pool,
        ):
            ident = singles.tile([P, P], F16)
            make_identity(nc, ident)

            def body(_iv=None):
                no_in_dma = variant in ("mm", "mm_t", "no_in_dma")
                no_out_dma = variant in ("mm", "mm_t", "no_out_dma")
                mm_only = variant in ("mm", "mm_t")

                def alloc_batch():
                    qt = qt_pool.tile([P, nh, tq], F16, tag="qt")
                    kt = kt_pool.tile([P, nh, tk], F16, tag="kt")
                    kn = kn_pool.tile([P, nk, h], F16, tag="kn")
                    return qt, kt, kn

                def load_strip(tiles_, b, i):
                    qt, kt, kn = tiles_
                    nc.sync.dma_start(out=qt[:, i, :], in_=qt_d[b, i * P : (i + 1) * P, :])
                    nc.sync.dma_start(out=kt[:, i, :], in_=kt_d[b, i * P : (i + 1) * P, :])
                    nc.sync.dma_start(out=kn[:, i, :], in_=kn_d[b, i * P : (i + 1) * P, :])

                def load_batch(b):
                    tiles_ = alloc_batch()
                    if not no_in_dma:
                        for i in range(nh):
                            load_strip(tiles_, b, i)
                    else:
                        qt, kt, kn = tiles_
                        nc.vector.memset(qt, 0.01)
                        nc.vector.memset(kt, 0.01)
                        nc.gpsimd.memset(kn, 0.01)
                    return tiles_

                def s_phase(qt_tile, kt, q0):
                    ps_s = ps_s_pool.tile([P, tk], F32, tag="ps_s")
                    negm_parts = []
                    for ci, (off, w) in enumerate(_mm_chunks(tk, chunk)):
                        for i in range(nh):
                            nc.tensor.matmul(
                                ps_s[:, off : off + w],
                                qt_tile[:, i, q0 : q0 + P],
                                kt[:, i, off : off + w],
                                start=(i == 0),
                                stop=(i == nh - 1),
                            )
                        # per-chunk -max so the row max is ready right when the
                        # last chunk's matmuls finish
                        if not mm_only:
                            nm = stats_pool.tile([P, 1], F32, tag=f"negm{ci}")
                            nc.vector.reduce_max(
                                out=nm,
                                in_=ps_s[:, off : off + w],
                                axis=mybir.AxisListType.X,
                                negate=True,
                            )
                            negm_parts.append(nm)
                    return ps_s, negm_parts

                def softmax_et(b, qt, ps_s, negm_parts):
                    if mm_only:
                        # timing ablation: skip softmax; E^T from memset ev
                        ett = et_pool.tile([P, nk, P], F16, tag="et")
                        r = None
                        if variant == "mm":
                            nc.gpsimd.memset(ett, 0.5)
                        if variant == "mm_t":
                            ev = e_pool.tile([P, tk], F16, tag="ev")
                            nc.gpsimd.memset(ev, 0.5)
                            for g, j0 in enumerate(range(0, nk, 4)):
                                jj = min(4, nk - j0)
                                pt = ps_t_pool.tile([P, 4, P], F16, tag="pt")
                                for j in range(j0, j0 + jj):
                                    nc.tensor.transpose(
                                        pt[:, j - j0, :], ev[:, j * P : (j + 1) * P], ident
                                    )
                                if g % 2 == 0:
                                    nc.vector.tensor_copy(ett[:, j0 : j0 + jj, :], pt[:, :jj, :])
                                else:
                                    nc.scalar.copy(ett[:, j0 : j0 + jj, :], pt[:, :jj, :])
                        return ett, r
                    if len(negm_parts) == 1:
                        negm = negm_parts[0]
                    else:
                        negm = stats_pool.tile([P, 1], F32, tag="negm")
                        nc.vector.tensor_tensor(
                            out=negm,
                            in0=negm_parts[0],
                            in1=negm_parts[1],
                            op=mybir.AluOpType.min,
                        )
                        for nm in negm_parts[2:]:
                            nc.vector.tensor_tensor(
                                out=negm, in0=negm, in1=nm, op=mybir.AluOpType.min
                            )
                    ev = e_pool.tile([P, tk], F16, tag="ev")
                    ett = et_pool.tile([P, nk, P], F16, tag="et")
                    esums = []
                    for ci, (off, w) in enumerate(_mm_chunks(tk, chunk)):
                        es = stats_pool.tile([P, 1], F32, tag=f"esum{ci}")
                        nc.vector.memset(es, 0.0)
                        nc.scalar.activation(
                            out=ev[:, off : off + w],
                            in_=ps_s[:, off : off + w],
                            func=mybir.ActivationFunctionType.Exp,
                            bias=negm,
                            scale=1.0,
                            accum_out=es,
                        )
                        esums.append(es)
                        if et_dma and et_split:
                            # E^T chunk via the DMA XBAR transpose (frees the
                            # PE of all transpose work), issued per exp chunk
                            nc.sync.dma_start_transpose(
                                out=ett[:, off // P : (off + w) // P, :],
                                in_=ev[:, off : off + w],
                            )
                    if et_dma and not et_split:
                        nc.sync.dma_start_transpose(out=ett, in_=ev)
                    if len(esums) == 1:
                        esum = esums[0]
                    else:
                        esum = stats_pool.tile([P, 1], F32, tag="esum")
                        nc.vector.tensor_add(esum, esums[0], esums[1])
                        for es in esums[2:]:
                            nc.vector.tensor_add(esum, esum, es)
                    # raw scores: PSUM -> SBUF staging -> DRAM
                    sout = s_pool.tile([P, tk], FOUT, tag="sout")
                    for ci, (off, w) in enumerate(_mm_chunks(tk, chunk)):
                        if ci % 2 == 0:
                            nc.vector.tensor_copy(
                                sout[:, off : off + w], ps_s[:, off : off + w]
                            )
                        else:
                            nc.scalar.copy(
                                sout[:, off : off + w], ps_s[:, off : off + w]
                            )
                    if not no_out_dma:
                        nc.gpsimd.dma_start(out=s_d[b, qt * P : (qt + 1) * P, :], in_=sout)
                    r = stats_pool.tile([P, 1], F32, tag="r")
                    nc.vector.reciprocal(r, esum)
                    if not et_dma:
                        # E^T via fp16 PE transposes (1 cycle/row)
                        for g, j0 in enumerate(range(0, nk, 4)):
                            jj = min(4, nk - j0)
                            pt = ps_t_pool.tile([P, 4, P], F16, tag="pt")
                            for j in range(j0, j0 + jj):
                                nc.tensor.transpose(
                                    pt[:, j - j0, :], ev[:, j * P : (j + 1) * P], ident
                                )
                            if g % 2 == 0:
                                nc.vector.tensor_copy(ett[:, j0 : j0 + jj, :], pt[:, :jj, :])
                            else:
                                nc.scalar.copy(ett[:, j0 : j0 + jj, :], pt[:, :jj, :])
                    return ett, r

                def c_phase(b, qt, ett, r, kn):
                    ps_c = ps_c_pool.tile([P, h], F32, tag="ps_c")
                    for off, w in _mm_chunks(h, chunk):
                        for j in range(nk):
                            nc.tensor.matmul(
                                ps_c[:, off : off + w],
                                ett[:, j, :],
                                kn[:, j, off : off + w],
                                start=(j == 0),
                                stop=(j == nk - 1),
                            )
                    if mm_only:
                        return
                    cout = c_pool.tile([P, h], FOUT, tag="cout")
                    for ci, (off, w) in enumerate(_mm_chunks(h, chunk)):
                        if ci % 2 == 0:
                            nc.vector.tensor_scalar_mul(
                                cout[:, off : off + w], ps_c[:, off : off + w], r
                            )
                        else:
                            nc.scalar.mul(cout[:, off : off + w], ps_c[:, off : off + w], r)
                    if not no_out_dma:
                        nc.gpsimd.dma_start(out=c_d[b, qt * P : (qt + 1) * P, :], in_=cout)

                tiles = [(b, qt) for b in range(b_pc) for qt in range(nq)]
                cur = load_batch(0)
                pending = None
                prev = None  # (b, qt, ps_s, negm_parts, kn) awaiting softmax
                prev2 = None  # (b, qt, ett, r, kn) awaiting C (deep mode only)
                for b, qt in tiles:
                    if qt == 0 and b > 0:
                        cur = pending
                        pending = None
                    if b + 1 < b_pc and not no_in_dma and spread:
                        # prefetch next batch one strip per q-tile
                        if qt == 0:
                            pending = alloc_batch()
                        load_strip(pending, b + 1, qt)
                    elif b + 1 < b_pc and qt == nq - 3:
                        # prefetch next batch while this one computes
                        pending = load_batch(b + 1)
                    ps_s, negm_parts = s_phase(cur[0], cur[1], qt * P)
                    if prev is not None:
                        pb, pqt, ps_s_prev, pnm, pkn = prev
                        ett, r = softmax_et(pb, pqt, ps_s_prev, pnm)
                        if deep:
                            if prev2 is not None:
                                c_phase(*prev2)
                            prev2 = (pb, pqt, ett, r, pkn)
                        else:
                            c_phase(pb, pqt, ett, r, pkn)
                    prev = (b, qt, ps_s, negm_parts, cur[2])
                pb, pqt, ps_s_prev, pnm, pkn = prev
                ett, r = softmax_et(pb, pqt, ps_s_prev, pnm)
                if deep:
                    if prev2 is not None:
                        c_phase(*prev2)
                    c_phase(pb, pqt, ett, r, pkn)
                else:
                    c_phase(pb, pqt, ett, r, pkn)

            if repeats == 1:
                body()
            else:
                with tc.For_i(
                    0, repeats, 1, hint_engines=(mybir.EngineType.PE,)
                ) as iv:
                    body(iv)

    nc.compile()
    return nc


_NC_CACHE = {}


def _get_nc(repeats=1, variant=None, chunk=512, et_dma=True, deep=False, out16=False,
            spread=False, et_split=False):
    key = (repeats, variant, chunk, et_dma, deep, out16, spread, et_split)
    if key not in _NC_CACHE:
        _NC_CACHE[key] = build_attention_nc(
            repeats=repeats, variant=variant, chunk=chunk, et_dma=et_dma, deep=deep,
            out16=out16, spread=spread, et_split=et_split,
        )
    return _NC_CACHE[key]


def _prep_inputs(query, memory_bank):
    q16 = np.asarray(query, dtype=np.float16)
    k16 = np.asarray(memory_bank, dtype=np.float16)
    qT = np.ascontiguousarray(q16.transpose(0, 2, 1))
    kT = np.ascontiguousarray(k16.transpose(0, 2, 1))
    return qT, kT, k16


def run_on_hw(query, memory_bank, repeats=1, **cfg):
    nc = _get_nc(repeats, **cfg)
    qT, kT, k16 = _prep_inputs(query, memory_bank)
    in_maps = [
        {
            "query_t": qT[c * B_PC : (c + 1) * B_PC],
            "memory_bank_t": kT[c * B_PC : (c + 1) * B_PC],
            "memory_bank_n": k16[c * B_PC : (c + 1) * B_PC],
        }
        for c in range(N_CORES)
    ]
    res = run_bass_kernel_spmd(nc, in_maps, core_ids=list(range(N_CORES)))
    context = np.concatenate([res.results[c]["context"] for c in range(N_CORES)], axis=0)
    scores = np.concatenate([res.results[c]["scores"] for c in range(N_CORES)], axis=0)
    if context.dtype != np.float32:
        context = context.astype(np.float32)
        scores = scores.astype(np.float32)
    return context, scores


def kernel(query, memory_bank):
    return run_on_hw(query, memory_bank, repeats=1)
